# revision 3
# baseline (speedup 1.0000x reference)
"""Trainium2 Bass kernel for nn_MetScore (histogram_binning).

Strategy (8 NeuronCores, data-parallel over the B*H*W pixel axis):
  Each core takes 1/8 of the 921600 pixels per timestep, for all T=20
  timesteps, laid out as [120 partitions = (t, g in 0..5), 19200 pixels]
  and processed in 8 chunks of 2400 pixels. Per chunk, fused
  elementwise+reduce instructions produce per-(t,g) partial sums:

    ScalarE (ACT, accum_out):  sum(30p), sum(30t), sum((30p)^2),
        sum((30t)^2), sum((MX-MN)^2), and sign-sums sum(sign(x-e)) that
        the host converts to step counts #{x>=e}.
    VectorE (DVE):  MN=min, MX=max, E=MX-MN (tensor_tensor);
        K_e=#{MN>=e} (tensor_scalar is_ge + add-accum);
        kx_e=[MX>=e] planes; KX_l=#{MN>=e_l & MX>=e_{l+1}} and
        D_e=sum(E*[Tm>=e]) via scalar_tensor_tensor + accum.

  All contingency-table stats derive from these on the host (exact
  integer counts), using min/max identities:
    hits_l   = K_{e_l} - KX_l          (both p,t in [e_l, e_{l+1}))
    cntP_l   = Sp_{e_l} - Sp_{e_{l+1}},  cntT_l analogous
    n_c      = Sp_0.1 + St_0.1 - K_0.1   (#{p>=.1 or t>=.1})
    mae_num_l= D_{e_l} - D_{e_{l+1}}
  Correlation moments over the not-double-zero set equal the all-pixel
  moments to ~1e-7 relative (double-zero pixels have p,t < 0.1, so
  their contribution is below fp32 noise of the reference itself), with
  sum(p*t) = (sum(p^2)+sum(t^2)-sum((p-t)^2))/2.

  Masked pixels are encoded as value -1.0 (so 30*x = -30), which drops
  them from every count/sum; n_valid = #{30p >= -15}. The final score
  combine runs on host in float64.
"""

import sys

sys.path.insert(0, "/opt/trn_rl_repo")

import numpy as np

B, T, H, W = 4, 20, 480, 480
N_CORES = 8
NPIX = B * H * W              # 921600 pixels per timestep
NPIX_CORE = NPIX // N_CORES   # 115200
G = 6                         # pixel groups per timestep -> 120 partitions
P = T * G                     # 120 partitions
FPP = NPIX_CORE // G          # 19200 pixels per partition row
NCH = 8                       # chunks
F = FPP // NCH                # 2400 chunk width

EDGES = [0.1, 1.0, 2.0, 5.0, 8.0]
NV_EDGE = -15.0               # valid pixels have 30*p >= 0 > -15; masked = -30

# ACT stat columns per chunk: 5 accums + 6 p-sign edges + 5 t-sign edges
SA_PER = 16
# DVE stat columns per chunk: K(5) + KX(4) + D(5)
SV_PER = 14

LEVEL_WEIGHTS = np.array([0.1, 0.1, 0.2, 0.25, 0.35], dtype=np.float64)
TIME_WEIGHTS = np.array(
    [0.0075, 0.02, 0.03, 0.04, 0.05, 0.06, 0.07, 0.08, 0.09, 0.1,
     0.09, 0.08, 0.07, 0.06, 0.05, 0.04, 0.03, 0.02, 0.0075, 0.005],
    dtype=np.float64,
)

_PROG = None  # built lazily once


def _split_multi_waits(nc, mybir, max_waits=1):
    """This toolchain's codegen rejects >1 sem wait per instruction; move
    extra waits onto standalone EventSemaphore instructions."""
    for fn in nc.m.functions:
        for blk in fn.blocks:
            insts = blk.instructions
            i = 0
            while i < len(insts):
                inst = insts[i]
                si = inst.sync_info
                if si is not None and len(si.on_wait) > max_waits:
                    waits = list(si.on_wait)
                    for w in waits[:-1]:
                        ev = mybir.InstEventSemaphore(
                            name=f"WSPLIT-{nc.next_id()}", ins=[], outs=[]
                        )
                        ev.engine = inst.engine
                        ev.sync_info = mybir.SyncInfo(on_wait=[w], on_update=[])
                        nc.register_instruction(ev)
                        blk.instructions.insert(i, ev)
                        i += 1
                    si.on_wait = waits[-1:]
                    inst.sync_info = si
                i += 1


def _build_program():
    import concourse.bass as bass
    import concourse.mybir as mybir
    import concourse.tile as tile

    AF = mybir.ActivationFunctionType
    OP = mybir.AluOpType
    f32 = mybir.dt.float32
    bf16 = mybir.dt.bfloat16

    nc = bass.Bass()

    # Register const APs for ACT Sign biases (bias = -edge).
    def reg_const(value):
        th = nc.alloc_sbuf_tensor(f"cst-{value}", [128, 1], f32)
        nc.gpsimd.memset(th.ap(), value)
        nc.const_aps.aps[(f32, value)] = th.ap()

    sign_p_edges = [NV_EDGE] + EDGES   # 6
    sign_t_edges = list(EDGES)         # 5
    for e in sorted(set(sign_p_edges + sign_t_edges)):
        reg_const(-e)
    nc.all_engine_barrier()

    p_in = nc.dram_tensor("p", [P, FPP], f32, kind="ExternalInput")
    t_in = nc.dram_tensor("t", [P, FPP], f32, kind="ExternalInput")
    sa_out = nc.dram_tensor("sa", [P, NCH * SA_PER], f32, kind="ExternalOutput")
    sv_out = nc.dram_tensor("sv", [P, NCH * SV_PER], f32, kind="ExternalOutput")

    with tile.TileContext(nc) as tc:
        with (
            tc.tile_pool(name="io", bufs=3) as io_pool,
            tc.tile_pool(name="pl", bufs=2) as plane_pool,
            tc.tile_pool(name="scr", bufs=1) as scr_pool,
            tc.tile_pool(name="psc", bufs=1, space="PSUM") as psum_pool,
            tc.tile_pool(name="st", bufs=1) as st_pool,
        ):
            sa = st_pool.tile([P, NCH * SA_PER], f32, tag="sa")
            sv = st_pool.tile([P, NCH * SV_PER], f32, tag="sv")

            for ch in range(NCH):
                sl = slice(ch * F, (ch + 1) * F)
                pt = io_pool.tile([P, F], f32, tag="p")
                tt = io_pool.tile([P, F], f32, tag="t")
                nc.sync.dma_start(pt[:], p_in[:, sl])
                nc.sync.dma_start(tt[:], t_in[:, sl])

                def sa_col(i):
                    return sa[:, ch * SA_PER + i : ch * SA_PER + i + 1]

                def sv_col(i):
                    return sv[:, ch * SV_PER + i : ch * SV_PER + i + 1]

                # --- ScalarE: scale to physical units + moment accums ---
                pm = plane_pool.tile([P, F], f32, tag="pm")
                tm = plane_pool.tile([P, F], f32, tag="tm")
                nc.scalar.activation(pm[:], pt[:], AF.Copy, scale=30.0,
                                     accum_out=sa_col(0))
                nc.scalar.activation(tm[:], tt[:], AF.Copy, scale=30.0,
                                     accum_out=sa_col(1))

                # --- VectorE: min/max/absdiff planes ---
                mn = plane_pool.tile([P, F], f32, tag="mn")
                mx = plane_pool.tile([P, F], f32, tag="mx")
                ee = plane_pool.tile([P, F], f32, tag="ee")
                nc.vector.tensor_tensor(mn[:], pm[:], tm[:], OP.min)
                nc.vector.tensor_tensor(mx[:], pm[:], tm[:], OP.max)
                nc.vector.tensor_tensor(ee[:], mx[:], mn[:], OP.subtract)

                # --- ScalarE: squares (fp32 scratch outs) ---
                sq = psum_pool.tile([P, F], f32, tag="sq")
                nc.scalar.activation(sq[:], pm[:], AF.Square, accum_out=sa_col(2))
                sq2 = psum_pool.tile([P, F], f32, tag="sq")
                nc.scalar.activation(sq2[:], tm[:], AF.Square, accum_out=sa_col(3))
                sq3 = psum_pool.tile([P, F], f32, tag="sq")
                nc.scalar.activation(sq3[:], ee[:], AF.Square, accum_out=sa_col(4))

                # --- ScalarE: sign-sums -> host converts to #{x>=e} ---
                for i, e in enumerate(sign_p_edges):
                    sg = psum_pool.tile([P, F], f32, tag="sq")
                    nc.scalar.activation(sg[:], pm[:], AF.Sign, bias=float(-e),
                                         accum_out=sa_col(5 + i))
                for i, e in enumerate(sign_t_edges):
                    sg = psum_pool.tile([P, F], f32, tag="sq")
                    nc.scalar.activation(sg[:], tm[:], AF.Sign, bias=float(-e),
                                         accum_out=sa_col(11 + i))

                # --- VectorE: K_e = #{MN>=e} (tensor_scalar 2x mode) ---
                for i, e in enumerate(EDGES):
                    ks = scr_pool.tile([P, F], bf16, tag="ks")
                    nc.vector.tensor_scalar(ks[:], mn[:], float(e), None,
                                            OP.is_ge, OP.add,
                                            accum_out=sv_col(i))

                # --- VectorE: kx planes + KX cross counts ---
                kxs = []
                for ei, e in enumerate(EDGES[1:]):
                    kx = scr_pool.tile([P, F], bf16, tag=f"kx{ei}")
                    nc.vector.tensor_scalar(kx[:], mx[:], float(e), None,
                                            OP.is_ge)
                    kxs.append(kx)
                for l in range(4):
                    ss = scr_pool.tile([P, F], bf16, tag="ss")
                    nc.vector.scalar_tensor_tensor(ss[:], mn[:], float(EDGES[l]),
                                                   kxs[l][:], OP.is_ge, OP.mult,
                                                   accum_out=sv_col(5 + l))

                # --- VectorE: D_e = sum(E * [Tm>=e]) ---
                for i, e in enumerate(EDGES):
                    ds = scr_pool.tile([P, F], bf16, tag="ds")
                    nc.vector.scalar_tensor_tensor(ds[:], tm[:], float(e),
                                                   ee[:], OP.is_ge, OP.mult,
                                                   accum_out=sv_col(9 + i))

            nc.sync.dma_start(sa_out[:], sa[:])
            nc.sync.dma_start(sv_out[:], sv[:])

    _split_multi_waits(nc, mybir)
    return nc


def _get_prog():
    global _PROG
    if _PROG is None:
        _PROG = _build_program()
    return _PROG


def _shard(x):
    """[B,T,H,W] f32 -> list of 8 arrays [P=120, FPP=19200], C-contiguous."""
    a = np.transpose(x, (1, 0, 2, 3)).reshape(T, NPIX)
    return [
        np.ascontiguousarray(
            a[:, c * NPIX_CORE : (c + 1) * NPIX_CORE].reshape(P, FPP)
        )
        for c in range(N_CORES)
    ]


def kernel(pred_norm, target_norm, mask):
    from concourse.bass_utils import run_bass_kernel_spmd

    p = np.asarray(pred_norm, dtype=np.float32)
    t = np.asarray(target_norm, dtype=np.float32)
    m = np.asarray(mask)
    if not m.all():
        # Encode masked pixels as -1.0 (-> 30x = -30): excluded from every
        # count/sum on device; host corrects the raw moment sums below.
        p = np.where(m, p, np.float32(-1.0))
        t = np.where(m, t, np.float32(-1.0))

    nc = _get_prog()
    in_maps = [
        {"p": pc, "t": tc_} for pc, tc_ in zip(_shard(p), _shard(t))
    ]
    res = run_bass_kernel_spmd(nc, in_maps, list(range(N_CORES)))

    # ---- host combine (float64) ----
    sa = np.zeros((T, SA_PER), dtype=np.float64)
    sv = np.zeros((T, SV_PER), dtype=np.float64)
    for r in res.results:
        sa += (
            r["sa"].astype(np.float64).reshape(T, G, NCH, SA_PER).sum(axis=(1, 2))
        )
        sv += (
            r["sv"].astype(np.float64).reshape(T, G, NCH, SV_PER).sum(axis=(1, 2))
        )

    sum_p, sum_t, sum_pp, sum_tt, sum_ee = (sa[:, i] for i in range(5))
    # sign-sums -> counts: #{x>=e} = (sum(sign(x-e)) + N) / 2
    npix = float(NPIX)
    Sp = {e: (sa[:, 5 + i] + npix) / 2.0 for i, e in enumerate([NV_EDGE] + EDGES)}
    St = {e: (sa[:, 11 + i] + npix) / 2.0 for i, e in enumerate(EDGES)}
    K = {e: sv[:, i] for i, e in enumerate(EDGES)}
    KX = [sv[:, 5 + l] for l in range(4)]
    D = {e: sv[:, 9 + i] for i, e in enumerate(EDGES)}

    n_valid = Sp[NV_EDGE]
    n_masked = npix - n_valid

    # correlation (over the not-double-zero set; masked-pixel sums removed)
    n_c = Sp[EDGES[0]] + St[EDGES[0]] - K[EDGES[0]]
    sum_p_c = sum_p + 30.0 * n_masked
    sum_t_c = sum_t + 30.0 * n_masked
    sum_pp_c = sum_pp - 900.0 * n_masked
    sum_tt_c = sum_tt - 900.0 * n_masked
    sum_pt_c = (sum_pp + sum_tt - sum_ee) / 2.0 - 900.0 * n_masked
    safe_n = np.maximum(n_c, 1.0)
    p_mean = sum_p_c / safe_n
    t_mean = sum_t_c / safe_n
    num = sum_pt_c - n_c * p_mean * t_mean
    den = np.sqrt(
        np.maximum(sum_pp_c - n_c * p_mean * p_mean, 0.0)
        * np.maximum(sum_tt_c - n_c * t_mean * t_mean, 0.0)
    )
    r_time = np.clip(num / (den + 1e-6), -1.0, 1.0)
    r_time = np.where(n_c > 0, r_time, 0.0)

    # contingency stats per level
    hi_edges = EDGES[1:] + [None]
    hits = np.stack(
        [K[EDGES[l]] - (KX[l] if l < 4 else 0.0) for l in range(5)], axis=1
    )
    cntP = np.stack(
        [Sp[EDGES[l]] - (Sp[hi_edges[l]] if hi_edges[l] else 0.0) for l in range(5)],
        axis=1,
    )
    cntT = np.stack(
        [St[EDGES[l]] - (St[hi_edges[l]] if hi_edges[l] else 0.0) for l in range(5)],
        axis=1,
    )
    misses = cntT - hits
    fas = cntP - hits
    ts = hits / (hits + misses + fas + 1e-8)

    mae_num = np.stack(
        [D[EDGES[l]] - (D[hi_edges[l]] if hi_edges[l] else 0.0) for l in range(5)],
        axis=1,
    )
    mae = np.where(cntT > 0, mae_num / np.maximum(cntT, 1.0), 0.0)

    any_valid = (n_valid > 0)[:, None]
    ts = np.where(any_valid, ts, 0.0)
    mae = np.where(any_valid, mae, 0.0)

    # combine
    term_corr = np.sqrt(np.exp(r_time - 1.0))
    term_mae = np.sqrt(np.exp(-mae / 100.0))
    sum_level = (LEVEL_WEIGHTS[None, :] * ts * term_mae).sum(-1)
    score_time = term_corr * sum_level
    total_score = (score_time * TIME_WEIGHTS).sum()

    f32 = np.float32
    return (
        f32(total_score),
        score_time.astype(f32),
        r_time.astype(f32),
        ts.astype(f32),
        mae.astype(f32),
        ts.mean(0).astype(f32),
        mae.mean(0).astype(f32),
    )


# revision 4
# speedup vs baseline: 7.9508x; 7.9508x over previous
"""Trainium2 Bass kernel for nn_MetScore (histogram_binning).

Strategy (8 NeuronCores, data-parallel over the B*H*W pixel axis):
  Each core takes 1/8 of the 921600 pixels per timestep, for all T=20
  timesteps, laid out as [120 partitions = (t, g in 0..5), 19200 pixels]
  and processed in 8 chunks of 2400 pixels. Per chunk, fused
  elementwise+reduce instructions produce per-(t,g) partial sums:

    ScalarE (ACT, accum_out):  sum(30p), sum(30t), sum((30p)^2),
        sum((30t)^2), sum((MX-MN)^2), and sign-sums sum(sign(x-e)) that
        the host converts to step counts #{x>=e}.
    VectorE (DVE):  MN=min, MX=max, E=MX-MN (tensor_tensor);
        K_e=#{MN>=e} (tensor_scalar is_ge + add-accum);
        kx_e=[MX>=e] planes; KX_l=#{MN>=e_l & MX>=e_{l+1}} and
        D_e=sum(E*[Tm>=e]) via scalar_tensor_tensor + accum.

  All contingency-table stats derive from these on the host (exact
  integer counts), using min/max identities:
    hits_l   = K_{e_l} - KX_l          (both p,t in [e_l, e_{l+1}))
    cntP_l   = Sp_{e_l} - Sp_{e_{l+1}},  cntT_l analogous
    n_c      = Sp_0.1 + St_0.1 - K_0.1   (#{p>=.1 or t>=.1})
    mae_num_l= D_{e_l} - D_{e_{l+1}}
  Correlation moments over the not-double-zero set equal the all-pixel
  moments to ~1e-7 relative (double-zero pixels have p,t < 0.1, so
  their contribution is below fp32 noise of the reference itself), with
  sum(p*t) = (sum(p^2)+sum(t^2)-sum((p-t)^2))/2.

  Masked pixels are encoded as value -1.0 (so 30*x = -30), which drops
  them from every count/sum; n_valid = #{30p >= -15}. The final score
  combine runs on host in float64.
"""

import sys

sys.path.insert(0, "/opt/trn_rl_repo")

import numpy as np

B, T, H, W = 4, 20, 480, 480
N_CORES = 8
NPIX = B * H * W              # 921600 pixels per timestep
NPIX_CORE = NPIX // N_CORES   # 115200
G = 6                         # pixel groups per timestep -> 120 partitions
P = T * G                     # 120 partitions
FPP = NPIX_CORE // G          # 19200 pixels per partition row
NCH = 8                       # chunks
F = FPP // NCH                # 2400 chunk width

EDGES = [0.1, 1.0, 2.0, 5.0, 8.0]
NV_EDGE = -15.0               # valid pixels have 30*p >= 0 > -15; masked = -30

# ACT stat columns per chunk: 5 accums + 6 p-sign edges + 5 t-sign edges
SA_PER = 16
# DVE stat columns per chunk: K(5) + KX(4) + D(5)
SV_PER = 14

LEVEL_WEIGHTS = np.array([0.1, 0.1, 0.2, 0.25, 0.35], dtype=np.float64)
TIME_WEIGHTS = np.array(
    [0.0075, 0.02, 0.03, 0.04, 0.05, 0.06, 0.07, 0.08, 0.09, 0.1,
     0.09, 0.08, 0.07, 0.06, 0.05, 0.04, 0.03, 0.02, 0.0075, 0.005],
    dtype=np.float64,
)

_PROG = {}  # built lazily once, keyed by repeat count


def _split_multi_waits(nc, mybir, max_waits=1):
    """This toolchain's codegen rejects >1 sem wait per instruction; move
    extra waits onto standalone EventSemaphore instructions."""
    for fn in nc.m.functions:
        for blk in fn.blocks:
            insts = blk.instructions
            i = 0
            while i < len(insts):
                inst = insts[i]
                si = inst.sync_info
                if si is not None and len(si.on_wait) > max_waits:
                    waits = list(si.on_wait)
                    for w in waits[:-1]:
                        ev = mybir.InstEventSemaphore(
                            name=f"WSPLIT-{nc.next_id()}", ins=[], outs=[]
                        )
                        ev.engine = inst.engine
                        ev.sync_info = mybir.SyncInfo(on_wait=[w], on_update=[])
                        nc.register_instruction(ev)
                        blk.instructions.insert(i, ev)
                        i += 1
                    si.on_wait = waits[-1:]
                    inst.sync_info = si
                i += 1


def _build_program(rep=1):
    import concourse.bass as bass
    import concourse.mybir as mybir
    import concourse.tile as tile

    AF = mybir.ActivationFunctionType
    OP = mybir.AluOpType
    f32 = mybir.dt.float32
    bf16 = mybir.dt.bfloat16

    nc = bass.Bass()

    # Register const APs for ACT Sign biases (bias = -edge).
    def reg_const(value):
        th = nc.alloc_sbuf_tensor(f"cst-{value}", [128, 1], f32)
        nc.gpsimd.memset(th.ap(), value)
        nc.const_aps.aps[(f32, value)] = th.ap()

    sign_p_edges = [NV_EDGE] + EDGES   # 6
    sign_t_edges = list(EDGES)         # 5
    for e in sorted(set(sign_p_edges + sign_t_edges)):
        reg_const(-e)
    nc.all_engine_barrier()

    p_in = nc.dram_tensor("p", [P, FPP], f32, kind="ExternalInput")
    t_in = nc.dram_tensor("t", [P, FPP], f32, kind="ExternalInput")
    sa_out = nc.dram_tensor("sa", [P, NCH * SA_PER], f32, kind="ExternalOutput")
    sv_out = nc.dram_tensor("sv", [P, NCH * SV_PER], f32, kind="ExternalOutput")

    with tile.TileContext(nc) as tc:
        with (
            tc.tile_pool(name="io", bufs=3) as io_pool,
            tc.tile_pool(name="pl", bufs=2) as plane_pool,
            tc.tile_pool(name="scr", bufs=1) as scr_pool,
            tc.tile_pool(name="psc", bufs=1, space="PSUM") as psum_pool,
            tc.tile_pool(name="st", bufs=1) as st_pool,
        ):
            sa = st_pool.tile([P, NCH * SA_PER], f32, tag="sa")
            sv = st_pool.tile([P, NCH * SV_PER], f32, tag="sv")

            for ch_rep in range(NCH * rep):
                ch = ch_rep % NCH
                sl = slice(ch * F, (ch + 1) * F)
                pt = io_pool.tile([P, F], f32, tag="p")
                tt = io_pool.tile([P, F], f32, tag="t")
                nc.sync.dma_start(pt[:], p_in[:, sl])
                nc.sync.dma_start(tt[:], t_in[:, sl])

                def sa_col(i):
                    return sa[:, ch * SA_PER + i : ch * SA_PER + i + 1]

                def sv_col(i):
                    return sv[:, ch * SV_PER + i : ch * SV_PER + i + 1]

                # --- ScalarE: scale to physical units + moment accums ---
                pm = plane_pool.tile([P, F], f32, tag="pm")
                tm = plane_pool.tile([P, F], f32, tag="tm")
                nc.scalar.activation(pm[:], pt[:], AF.Copy, scale=30.0,
                                     accum_out=sa_col(0))
                nc.scalar.activation(tm[:], tt[:], AF.Copy, scale=30.0,
                                     accum_out=sa_col(1))

                # --- VectorE: min/max/absdiff planes ---
                mn = plane_pool.tile([P, F], f32, tag="mn")
                mx = plane_pool.tile([P, F], f32, tag="mx")
                ee = plane_pool.tile([P, F], f32, tag="ee")
                nc.vector.tensor_tensor(mn[:], pm[:], tm[:], OP.min)
                nc.vector.tensor_tensor(mx[:], pm[:], tm[:], OP.max)
                nc.vector.tensor_tensor(ee[:], mx[:], mn[:], OP.subtract)

                # --- ScalarE: squares (fp32 scratch outs) ---
                sq = psum_pool.tile([P, F], f32, tag="sq")
                nc.scalar.activation(sq[:], pm[:], AF.Square, accum_out=sa_col(2))
                sq2 = psum_pool.tile([P, F], f32, tag="sq")
                nc.scalar.activation(sq2[:], tm[:], AF.Square, accum_out=sa_col(3))
                sq3 = psum_pool.tile([P, F], f32, tag="sq")
                nc.scalar.activation(sq3[:], ee[:], AF.Square, accum_out=sa_col(4))

                # --- ScalarE: sign-sums -> host converts to #{x>=e} ---
                for i, e in enumerate(sign_p_edges):
                    sg = psum_pool.tile([P, F], f32, tag="sq")
                    nc.scalar.activation(sg[:], pm[:], AF.Sign, bias=float(-e),
                                         accum_out=sa_col(5 + i))
                for i, e in enumerate(sign_t_edges):
                    sg = psum_pool.tile([P, F], f32, tag="sq")
                    nc.scalar.activation(sg[:], tm[:], AF.Sign, bias=float(-e),
                                         accum_out=sa_col(11 + i))

                # --- VectorE: K_e = #{MN>=e} (tensor_scalar 2x mode) ---
                for i, e in enumerate(EDGES):
                    ks = scr_pool.tile([P, F], bf16, tag="ks")
                    nc.vector.tensor_scalar(ks[:], mn[:], float(e), None,
                                            OP.is_ge, OP.add,
                                            accum_out=sv_col(i))

                # --- VectorE: kx planes + KX cross counts ---
                kxs = []
                for ei, e in enumerate(EDGES[1:]):
                    kx = scr_pool.tile([P, F], bf16, tag=f"kx{ei}")
                    nc.vector.tensor_scalar(kx[:], mx[:], float(e), None,
                                            OP.is_ge)
                    kxs.append(kx)
                for l in range(4):
                    ss = scr_pool.tile([P, F], bf16, tag="ss")
                    nc.vector.scalar_tensor_tensor(ss[:], mn[:], float(EDGES[l]),
                                                   kxs[l][:], OP.is_ge, OP.mult,
                                                   accum_out=sv_col(5 + l))

                # --- VectorE: D_e = sum(E * [Tm>=e]) ---
                for i, e in enumerate(EDGES):
                    ds = scr_pool.tile([P, F], bf16, tag="ds")
                    nc.vector.scalar_tensor_tensor(ds[:], tm[:], float(e),
                                                   ee[:], OP.is_ge, OP.mult,
                                                   accum_out=sv_col(9 + i))

            nc.sync.dma_start(sa_out[:], sa[:])
            nc.sync.dma_start(sv_out[:], sv[:])

    _split_multi_waits(nc, mybir)
    return nc


def _get_prog(rep=1):
    if rep not in _PROG:
        _PROG[rep] = _build_program(rep)
    return _PROG[rep]


def _shard(x):
    """[B,T,H,W] f32 -> list of 8 arrays [P=120, FPP=19200], C-contiguous."""
    a = np.transpose(x, (1, 0, 2, 3)).reshape(T, NPIX)
    return [
        np.ascontiguousarray(
            a[:, c * NPIX_CORE : (c + 1) * NPIX_CORE].reshape(P, FPP)
        )
        for c in range(N_CORES)
    ]


def kernel(pred_norm, target_norm, mask):
    from concourse.bass_utils import run_bass_kernel_spmd

    p = np.asarray(pred_norm, dtype=np.float32)
    t = np.asarray(target_norm, dtype=np.float32)
    m = np.asarray(mask)
    if not m.all():
        # Encode masked pixels as -1.0 (-> 30x = -30): excluded from every
        # count/sum on device; host corrects the raw moment sums below.
        p = np.where(m, p, np.float32(-1.0))
        t = np.where(m, t, np.float32(-1.0))

    nc = _get_prog()
    in_maps = [
        {"p": pc, "t": tc_} for pc, tc_ in zip(_shard(p), _shard(t))
    ]
    res = run_bass_kernel_spmd(nc, in_maps, list(range(N_CORES)))

    # ---- host combine (float64) ----
    sa = np.zeros((T, SA_PER), dtype=np.float64)
    sv = np.zeros((T, SV_PER), dtype=np.float64)
    for r in res.results:
        sa += (
            r["sa"].astype(np.float64).reshape(T, G, NCH, SA_PER).sum(axis=(1, 2))
        )
        sv += (
            r["sv"].astype(np.float64).reshape(T, G, NCH, SV_PER).sum(axis=(1, 2))
        )

    sum_p, sum_t, sum_pp, sum_tt, sum_ee = (sa[:, i] for i in range(5))
    # sign-sums -> counts: #{x>=e} = (sum(sign(x-e)) + N) / 2
    npix = float(NPIX)
    Sp = {e: (sa[:, 5 + i] + npix) / 2.0 for i, e in enumerate([NV_EDGE] + EDGES)}
    St = {e: (sa[:, 11 + i] + npix) / 2.0 for i, e in enumerate(EDGES)}
    K = {e: sv[:, i] for i, e in enumerate(EDGES)}
    KX = [sv[:, 5 + l] for l in range(4)]
    D = {e: sv[:, 9 + i] for i, e in enumerate(EDGES)}

    n_valid = Sp[NV_EDGE]
    n_masked = npix - n_valid

    # correlation (over the not-double-zero set; masked-pixel sums removed)
    n_c = Sp[EDGES[0]] + St[EDGES[0]] - K[EDGES[0]]
    sum_p_c = sum_p + 30.0 * n_masked
    sum_t_c = sum_t + 30.0 * n_masked
    sum_pp_c = sum_pp - 900.0 * n_masked
    sum_tt_c = sum_tt - 900.0 * n_masked
    sum_pt_c = (sum_pp + sum_tt - sum_ee) / 2.0 - 900.0 * n_masked
    safe_n = np.maximum(n_c, 1.0)
    p_mean = sum_p_c / safe_n
    t_mean = sum_t_c / safe_n
    num = sum_pt_c - n_c * p_mean * t_mean
    den = np.sqrt(
        np.maximum(sum_pp_c - n_c * p_mean * p_mean, 0.0)
        * np.maximum(sum_tt_c - n_c * t_mean * t_mean, 0.0)
    )
    r_time = np.clip(num / (den + 1e-6), -1.0, 1.0)
    r_time = np.where(n_c > 0, r_time, 0.0)

    # contingency stats per level
    hi_edges = EDGES[1:] + [None]
    hits = np.stack(
        [K[EDGES[l]] - (KX[l] if l < 4 else 0.0) for l in range(5)], axis=1
    )
    cntP = np.stack(
        [Sp[EDGES[l]] - (Sp[hi_edges[l]] if hi_edges[l] else 0.0) for l in range(5)],
        axis=1,
    )
    cntT = np.stack(
        [St[EDGES[l]] - (St[hi_edges[l]] if hi_edges[l] else 0.0) for l in range(5)],
        axis=1,
    )
    misses = cntT - hits
    fas = cntP - hits
    ts = hits / (hits + misses + fas + 1e-8)

    mae_num = np.stack(
        [D[EDGES[l]] - (D[hi_edges[l]] if hi_edges[l] else 0.0) for l in range(5)],
        axis=1,
    )
    mae = np.where(cntT > 0, mae_num / np.maximum(cntT, 1.0), 0.0)

    any_valid = (n_valid > 0)[:, None]
    ts = np.where(any_valid, ts, 0.0)
    mae = np.where(any_valid, mae, 0.0)

    # combine
    term_corr = np.sqrt(np.exp(r_time - 1.0))
    term_mae = np.sqrt(np.exp(-mae / 100.0))
    sum_level = (LEVEL_WEIGHTS[None, :] * ts * term_mae).sum(-1)
    score_time = term_corr * sum_level
    total_score = (score_time * TIME_WEIGHTS).sum()

    f32 = np.float32
    return (
        f32(total_score),
        score_time.astype(f32),
        r_time.astype(f32),
        ts.astype(f32),
        mae.astype(f32),
        ts.mean(0).astype(f32),
        mae.mean(0).astype(f32),
    )
